# revision 1
# baseline (speedup 1.0000x reference)
"""2-layer GAT (graph attention) Bass kernel for trn2, 8-core SPMD.

Sharding: query-node dim N row-sharded across cores. Per layer:
  1. each core computes Wh = x @ W for its node shard (all heads) on PE
     (fp32r), plus attention score vectors s1 (own queries) / s2 (own keys),
  2. AllGather of (Wh | s2) so every core has all keys/values,
  3. each core computes its rows of e/softmax/aggregation:
       scores transposed layout [j(keys)=partitions, q(queries)=free] so the
       softmax denominator is a PE ones-matmul and the aggregation needs no
       transposes: U[d,q] = sum_j Wh[j,d] * pm[j,q] with lhsT=Wh, rhs=pm.
     adjacency mask folded in as an additive -1e30 bias before leaky-relu.
  4. epilogue: h' = U / Z, double-ELU (reference applies elu twice), feeding
     layer 2 in the transposed [feature, node] layout the Wh matmul wants.

PSUM plan (8 banks, tags bankA0..5 / bankB0..1, all [128, Q]):
  Wh phase: pw chunks on A0..A3 (double buffered), s2 on A4, s1 on A5,
  bcast transients on B0/B1; main loop: U[h] on A{h}, Z rows 32-aligned in
  B0/B1; epilogue: 1/Z row tiles then per-head U/Z scale transients on B.

Output per core: [H, D, Q] (head, feature, local query) + host reassembly.
"""
import sys
sys.path.insert(0, "/opt/trn_rl_repo")

import numpy as np
import ml_dtypes
import concourse.bass as bass
import concourse.mybir as mybir
import concourse.tile as tile
from concourse import bacc

F32 = mybir.dt.float32
F32R = mybir.dt.float32r
ALU = mybir.AluOpType
AF = mybir.ActivationFunctionType

MASK_BIG = 1000.0


class Cfg:
    def __init__(self, N=4096, F=768, H=6, D=128, n_cores=8, use_prelu=False):
        self.N, self.F, self.H, self.D, self.n_cores = N, F, H, D, n_cores
        self.use_prelu = use_prelu
        assert D == 128
        assert F % 128 == 0 and N % (128 * n_cores) == 0
        assert H <= 6
        self.Q = N // n_cores          # queries per core
        self.NB = self.Q // 128        # query blocks per core
        self.JB = N // 128             # key blocks total
        self.FB = F // 128             # feature blocks
        self.HD = H * D                # concat feature dim
        # heads per exp-pack (free dim of one ACT exp instruction)
        self.PACK = 3 if H % 3 == 0 else (2 if H % 2 == 0 else 1)


def _col_chunks(total, step=512):
    return [(c0, min(c0 + step, total)) for c0 in range(0, total, step)]


def build_gat_nc(cfg: Cfg, debug=False, fake_ag=False):
    c = cfg
    nc = bacc.Bacc("TRN2", target_bir_lowering=False, debug=debug,
                   num_devices=1 if fake_ag else c.n_cores)

    # ---- I/O ----
    xT_s = nc.dram_tensor("xT_s", [c.F, c.Q], F32, kind="ExternalInput")
    mb_in = nc.dram_tensor("mb", [c.N, c.Q], mybir.dt.bfloat16,
                           kind="ExternalInput")
    w1cat = nc.dram_tensor("w1cat", [c.F, c.HD], F32, kind="ExternalInput")
    w2cat = nc.dram_tensor("w2cat", [c.HD, c.HD], F32, kind="ExternalInput")
    wa_in = {}
    for nm in ("wa1f", "wa1s"):
        wa_in[nm] = nc.dram_tensor(nm, [c.F, c.H], F32, kind="ExternalInput")
    for nm in ("wa2f", "wa2s"):
        wa_in[nm] = nc.dram_tensor(nm, [c.HD, c.H], F32, kind="ExternalInput")
    w2colsum = nc.dram_tensor("w2colsum", [1, c.HD], F32, kind="ExternalInput")
    wa2fcs = nc.dram_tensor("wa2fcs", [1, c.H], F32, kind="ExternalInput")
    wa2scs = nc.dram_tensor("wa2scs", [1, c.H], F32, kind="ExternalInput")
    out_g = nc.dram_tensor("out_g", [c.H, c.D, c.Q], F32, kind="ExternalOutput")

    with tile.TileContext(nc) as tc:
        import contextlib
        ctx = contextlib.ExitStack()
        with ctx:
            const = ctx.enter_context(tc.tile_pool(name="const", bufs=1))
            mbpool = ctx.enter_context(tc.tile_pool(name="mbpool", bufs=6))
            xpool = ctx.enter_context(tc.tile_pool(name="xpool", bufs=1))
            stage = ctx.enter_context(tc.tile_pool(name="stage", bufs=2))
            wcpool = ctx.enter_context(tc.tile_pool(name="wcpool", bufs=1))
            wapool = ctx.enter_context(tc.tile_pool(name="wapool", bufs=1))
            whpool = ctx.enter_context(tc.tile_pool(name="whpool", bufs=1))
            srow = ctx.enter_context(tc.tile_pool(name="srow", bufs=1))
            bcpool = ctx.enter_context(tc.tile_pool(name="bcpool", bufs=1))
            wtpool = ctx.enter_context(tc.tile_pool(name="wtpool", bufs=12))
            ztpool = ctx.enter_context(tc.tile_pool(name="ztpool", bufs=4))
            packpool = ctx.enter_context(tc.tile_pool(name="packpool", bufs=3))
            epil = ctx.enter_context(tc.tile_pool(name="epil", bufs=2))
            x2pool = ctx.enter_context(tc.tile_pool(name="x2pool", bufs=1))
            pp = ctx.enter_context(tc.tile_pool(name="pp", bufs=1, space="PSUM"))
            dram = ctx.enter_context(tc.tile_pool(name="dram", bufs=1, space="DRAM"))

            # ---- constants ----
            ones1r = const.tile([1, 128], F32, tag="c2")
            nc.any.memset(ones1r[:], 1.0)
            onescf = const.tile([128, 1], F32, tag="c3")
            nc.any.memset(onescf[:], 1.0)
            onescr = const.tile([128, 1], F32R, tag="c4")
            nc.vector.tensor_copy(onescr[:], onescf[:])
            ones32f = const.tile([128, 32], F32, tag="c16")
            nc.any.memset(ones32f[:], 1.0)
            ones32r = const.tile([128, 32], F32R, tag="c17")
            nc.vector.tensor_copy(ones32r[:], ones32f[:])
            monescol = const.tile([128, 1], F32, tag="c15")
            nc.any.memset(monescol[:], -1.0)
            mones1r = const.tile([1, 128], F32, tag="c6")
            nc.any.memset(mones1r[:], -1.0)
            onesQr = const.tile([1, c.Q], F32, tag="c10")
            nc.any.memset(onesQr[:], 1.0)
            w2csr = const.tile([1, c.HD], F32, tag="c8")
            nc.sync.dma_start(w2csr[:], w2colsum[:])
            wa2fcs_r = const.tile([1, c.H], F32, tag="c12")
            nc.sync.dma_start(wa2fcs_r[:], wa2fcs[:])
            wa2scs_r = const.tile([1, c.H], F32, tag="c14")
            nc.sync.dma_start(wa2scs_r[:], wa2scs[:])

            # ---- layer-1 lhsT tiles: xT shard, converted to f32r ----
            x1t = []
            for fb in range(c.FB):
                tr = xpool.tile([128, c.Q], F32, tag=f"x1r{fb}")
                nc.sync.dma_start(tr[:], xT_s[fb * 128:(fb + 1) * 128, :])
                x1t.append(tr)

            def load_wa(name, rows):
                tiles = []
                for fb in range(rows // 128):
                    tr = wapool.tile([128, c.H], F32, tag=f"{name}r{fb}")
                    nc.sync.dma_start(tr[:], wa_in[name][fb * 128:(fb + 1) * 128, :])
                    tiles.append(tr)
                return tiles

            wa1f_t = load_wa("wa1f", c.F)
            wa1s_t = load_wa("wa1s", c.F)
            wa2f_t = load_wa("wa2f", c.HD)
            wa2s_t = load_wa("wa2s", c.HD)

            def load_wcat(src, rows):
                tiles = []
                for fb in range(rows // 128):
                    tr = wcpool.tile([128, c.HD], F32, tag=f"wcr{fb}")
                    nc.sync.dma_start(tr[:], src[fb * 128:(fb + 1) * 128, :])
                    tiles.append(tr)
                return tiles

            chunks = _col_chunks(c.HD)
            W129 = c.D + 1

            def gat_layer(lname, xtiles, wcat_src, n_in_blocks, waf, was,
                          add_colsum, out_f32r):
                """One GAT layer. xtiles: list of [128, Q] f32r lhsT tiles
                (transposed input features). Returns list of H tiles
                [128(D), Q] (f32r if out_f32r) of g = elu2(h') + 1."""
                wct = load_wcat(wcat_src, n_in_blocks * 128)

                # --- Wh shard + s2 (per-query layout) ---
                whsb = []   # NB tiles [128, H*(D+1)] : per head D cols + s2 col
                for nb in range(c.NB):
                    a0 = (nb % 2) * 2
                    pwc = []
                    for ci, (c0, c1) in enumerate(chunks):
                        t = pp.tile([128, c1 - c0], F32, tag=f"pslot{a0 + ci}")
                        pwc.append(t)
                    for fb in range(n_in_blocks):
                        st, sp = (fb == 0), (fb == n_in_blocks - 1)
                        for ci, (c0, c1) in enumerate(chunks):
                            nc.tensor.matmul(pwc[ci][:],
                                             xtiles[fb][:, nb * 128:(nb + 1) * 128],
                                             wct[fb][:, c0:c1], start=st,
                                             stop=sp and not add_colsum)
                    if add_colsum:
                        for ci, (c0, c1) in enumerate(chunks):
                            nc.tensor.matmul(pwc[ci][:], mones1r[:],
                                             w2csr[:, c0:c1],
                                             start=False, stop=True)
                    # s2 for this query block: [128, H]
                    ps2 = pp.tile([128, c.H], F32, tag="pslot4")
                    for fb in range(n_in_blocks):
                        nc.tensor.matmul(ps2[:], xtiles[fb][:, nb * 128:(nb + 1) * 128],
                                         was[fb], start=(fb == 0),
                                         stop=(fb == n_in_blocks - 1) and not add_colsum)
                    if add_colsum:
                        nc.tensor.matmul(ps2[:], ones1r[:], wa2scs_r[:],
                                         start=False, stop=True)
                    # evict Wh + s2 interleaved as [h*(D+1)] with s2 in
                    # col D, split into per-head-group tiles so each group's
                    # AllGather can fire independently
                    GRP_ = c.PACK
                    wsbg = []
                    for g in range(c.H // GRP_):
                        wt_ = whpool.tile([128, GRP_ * W129], F32R,
                                          tag=f"whsb{nb}g{g}")
                        wsbg.append(wt_)
                    for h in range(c.H):
                        g, i = divmod(h, GRP_)
                        src_c = h * c.D
                        ci = src_c // 512
                        off = src_c - ci * 512
                        nc.scalar.copy(wsbg[g][:, i * W129:i * W129 + c.D],
                                       pwc[ci][:, off:off + c.D])
                        nc.scalar.copy(wsbg[g][:, i * W129 + c.D:(i + 1) * W129],
                                       ps2[:, h:h + 1])
                    whsb.append(wsbg)

                # --- s1 rows [H, Q] -> per-head [1, Q] f32r rows ---
                ps1 = pp.tile([c.H, c.Q], F32, tag="pslot5")
                for fb in range(n_in_blocks):
                    nc.tensor.matmul(ps1[:], waf[fb], xtiles[fb],
                                     start=(fb == 0),
                                     stop=(fb == n_in_blocks - 1) and not add_colsum)
                if add_colsum:
                    nc.tensor.matmul(ps1[:], wa2fcs_r[:], onesQr[:],
                                     start=False, stop=True)
                s1all = srow.tile([c.H, c.Q], F32, tag="s1all")
                nc.vector.tensor_copy(s1all[:], ps1[:])
                s1rows = []
                for h in range(c.H):
                    sr = srow.tile([1, c.Q], F32, tag=f"row{h}")
                    nc.sync.dma_start(sr[:], s1all[h:h + 1, :])
                    s1rows.append(sr)

                # --- pack to DRAM + one AllGather per head-group ---
                GRPW = c.PACK * W129
                ag_outs = []
                for g in range(c.H // c.PACK):
                    ag_in = dram.tile([c.Q, GRPW], F32R, tag=f"{lname}agin{g}")
                    for nb in range(c.NB):
                        nc.sync.dma_start(ag_in[nb * 128:(nb + 1) * 128, :],
                                          whsb[nb][g][:])
                    ag_out = dram.tile([c.N, GRPW], F32R, tag=f"{lname}agout{g}",
                                       addr_space="Shared" if c.n_cores > 4
                                       else "Local")
                    if fake_ag:
                        nc.sync.dma_start(ag_out[0:c.Q, :], ag_in[:])
                    else:
                        nc.gpsimd.collective_compute(
                            "AllGather", ALU.bypass,
                            replica_groups=[list(range(c.n_cores))],
                            ins=[ag_in[:].opt()], outs=[ag_out[:].opt()])
                    ag_outs.append(ag_out)

                # --- per-head-group attention: GRP heads at a time ---
                GRP = c.PACK
                ngrp = c.H // GRP
                gout = [None] * c.H
                for grp in range(ngrp):
                    heads = list(range(grp * GRP, (grp + 1) * GRP))
                    # broadcast tiles bc[h] = s1[h, :] replicated on partitions
                    bct = {}
                    for i, h in enumerate(heads):
                        pb = pp.tile([128, c.Q], F32, tag=f"pslot{6 + i % 2}")
                        nc.tensor.matmul(pb[:], ones1r[:], s1rows[h][:],
                                         start=True, stop=True)
                        bc = bcpool.tile([128, c.Q], F32, tag=f"bc{i}")
                        nc.scalar.copy(bc[:], pb[:])
                        bct[h] = bc
                    up = {}
                    zps = {}
                    for i, h in enumerate(heads):
                        upt = pp.tile([128, c.Q], F32, tag=f"pslot{i}")
                        up[h] = upt
                        zpt = pp.tile([1, c.Q], F32, tag=f"pslot{3 + i}")
                        zps[h] = zpt
                    for jb in range(c.JB):
                        mbt = mbpool.tile([128, c.Q], mybir.dt.bfloat16, tag="mb")
                        nc.sync.dma_start(mbt[:], mb_in[jb * 128:(jb + 1) * 128, :])
                        wtg = wtpool.tile([128, GRP * W129], F32R, tag="wt")
                        nc.sync.dma_start(
                            wtg[:], ag_outs[grp][jb * 128:(jb + 1) * 128, :])
                        wt = {h: wtg[:, i * W129:(i + 1) * W129]
                              for i, h in enumerate(heads)}
                        ppk = packpool.tile([128, GRP * c.Q], F32R, tag="ppack")
                        zpk = ztpool.tile([128, GRP * c.Q], F32, tag="zpack")
                        act_lrelu = c.use_prelu and (
                            c.use_prelu != "hybrid" or jb % 3 != 0)
                        for sl, h in enumerate(heads):
                            s2col = wt[h][:, c.D:W129].bitcast(F32)
                            if act_lrelu:
                                nc.vector.scalar_tensor_tensor(
                                    zpk[:, sl * c.Q:(sl + 1) * c.Q],
                                    bct[h][:], s2col, mbt[:], ALU.add, ALU.add)
                                if sl == GRP - 1:
                                    tpk = ztpool.tile([128, GRP * c.Q], F32,
                                                      tag="tpack")
                                    nc.scalar.activation(tpk[:], zpk[:], AF.Prelu,
                                                         alpha=0.2)
                                    nc.scalar.activation(ppk[:], tpk[:], AF.Exp)
                            else:
                                zt = ztpool.tile([128, c.Q], F32, tag="zt")
                                nc.vector.scalar_tensor_tensor(
                                    zt[:], bct[h][:], s2col, mbt[:],
                                    ALU.add, ALU.add)
                                nc.vector.scalar_tensor_tensor(
                                    zpk[:, sl * c.Q:(sl + 1) * c.Q],
                                    zt[:], 0.2, zt[:], ALU.mult, ALU.max)
                                if sl == GRP - 1:
                                    nc.scalar.activation(ppk[:], zpk[:], AF.Exp)
                        for sl, h in enumerate(heads):
                            pmslice = ppk[:, sl * c.Q:(sl + 1) * c.Q]
                            nc.tensor.matmul(up[h][:], wt[h][:, 0:c.D], pmslice,
                                             start=(jb == 0), stop=(jb == c.JB - 1))
                            nc.tensor.matmul(zps[h][:], onescr[:], pmslice,
                                             start=(jb == 0), stop=(jb == c.JB - 1))

                    # epilogue for this group: h' = U/Z, g = elu2(h') + 1
                    rzrows = {}
                    for h in heads:
                        rzr_h = srow.tile([1, c.Q], F32, tag=f"row{h}")
                        nc.vector.reciprocal(rzr_h[:], zps[h][:])
                        rzrows[h] = rzr_h
                    for i, h in enumerate(heads):
                        prb = pp.tile([128, c.Q], F32, tag=f"pslot{6 + i % 2}")
                        nc.tensor.matmul(prb[:], ones1r[:], rzrows[h][:],
                                         start=True, stop=True)
                        rb = epil.tile([128, c.Q], F32, tag="rb")
                        nc.scalar.copy(rb[:], prb[:])
                        xn = epil.tile([128, c.Q], F32, tag="xn")
                        nc.vector.tensor_tensor(xn[:], up[h][:], rb[:], ALU.mult)
                        mt = epil.tile([128, c.Q], F32, tag="mt")
                        nc.vector.tensor_scalar_min(mt[:], xn[:], 0.0)
                        e1 = epil.tile([128, c.Q], F32, tag="e1")
                        nc.scalar.activation(e1[:], mt[:], AF.Exp)
                        e2 = epil.tile([128, c.Q], F32, tag="e2")
                        nc.scalar.activation(e2[:], e1[:], AF.Exp, bias=monescol[:])
                        r1 = epil.tile([128, c.Q], F32, tag="r1")
                        nc.scalar.activation(r1[:], xn[:], AF.Relu)
                        g = x2pool.tile([128, c.Q], F32, tag=f"g{h}")
                        nc.vector.tensor_tensor(g[:], r1[:], e2[:], ALU.add)
                        gout[h] = g
                return gout

            x2t = gat_layer("L1", x1t, w1cat, c.FB, wa1f_t, wa1s_t,
                            add_colsum=False, out_f32r=True)
            gfin = gat_layer("L2", x2t, w2cat, c.HD // 128, wa2f_t, wa2s_t,
                             add_colsum=True, out_f32r=False)

            for h in range(c.H):
                nc.sync.dma_start(out_g[h, :, :], gfin[h][:])

    nc.compile()
    return nc


def host_prep(cfg: Cfg, x, adj, W1, a1, W2, a2):
    """Build per-core input maps from the full problem inputs."""
    c = cfg
    xT = np.ascontiguousarray(x.T)                               # [F, N]
    adjT = adj.T.astype(np.float32)                              # [N, N] (j,q)
    W1cat = np.ascontiguousarray(W1.transpose(1, 0, 2).reshape(c.F, c.HD))
    W2cat = np.ascontiguousarray(W2.transpose(1, 0, 2).reshape(c.HD, c.HD))
    wa1f = np.stack([W1[h].astype(np.float64) @ a1[h, :c.D, 0].astype(np.float64)
                     for h in range(c.H)], axis=1).astype(np.float32)  # [F, H]
    wa1s = np.stack([W1[h].astype(np.float64) @ a1[h, c.D:, 0].astype(np.float64)
                     for h in range(c.H)], axis=1).astype(np.float32)
    wa2f = np.stack([W2[h].astype(np.float64) @ a2[h, :c.D, 0].astype(np.float64)
                     for h in range(c.H)], axis=1).astype(np.float32)  # [HD, H]
    wa2s = np.stack([W2[h].astype(np.float64) @ a2[h, c.D:, 0].astype(np.float64)
                     for h in range(c.H)], axis=1).astype(np.float32)
    w2colsum = W2cat.sum(axis=0, keepdims=True).astype(np.float32)
    wa2fcs = -wa2f.sum(axis=0, keepdims=True).astype(np.float32)   # [1, H]
    wa2scs = -wa2s.sum(axis=0, keepdims=True).astype(np.float32)
    in_maps = []
    for cid in range(c.n_cores):
        qs, qe = cid * c.Q, (cid + 1) * c.Q
        mb = ((adjT[:, qs:qe] - 1.0) * MASK_BIG).astype(ml_dtypes.bfloat16)
        in_maps.append({
            "xT_s": np.ascontiguousarray(xT[:, qs:qe]),
            "mb": np.ascontiguousarray(mb),
            "w1cat": W1cat, "w2cat": W2cat,
            "wa1f": wa1f, "wa1s": wa1s, "wa2f": wa2f, "wa2s": wa2s,
            "w2colsum": w2colsum, "wa2fcs": wa2fcs, "wa2scs": wa2scs,
        })
    return in_maps


def host_finish(cfg: Cfg, results):
    """Assemble full [N, H*D] output from per-core out_g [H, D, Q]."""
    c = cfg
    out = np.empty((c.N, c.HD), np.float32)
    for cid in range(c.n_cores):
        g = results[cid]["out_g"]                      # [H, D, Q]
        out[cid * c.Q:(cid + 1) * c.Q, :] = (
            g.transpose(2, 0, 1).reshape(c.Q, c.HD) - 1.0)
    return out


# ---------------------------------------------------------------------------
# harness entry point: full-size problem, 8 cores
# ---------------------------------------------------------------------------
_NC_CACHE = {}


def _get_nc(cfg: Cfg):
    key = (cfg.N, cfg.F, cfg.H, cfg.D, cfg.n_cores, cfg.use_prelu)
    if key not in _NC_CACHE:
        _NC_CACHE[key] = build_gat_nc(cfg, debug=False)
    return _NC_CACHE[key]


def kernel(x, adj, W1, a1, W2, a2):
    """Full-input entry: shards across 8 NeuronCores internally."""
    from concourse import bass_utils
    cfg = Cfg(N=4096, F=768, H=6, D=128, n_cores=8, use_prelu="hybrid")
    assert x.shape == (cfg.N, cfg.F) and adj.shape == (cfg.N, cfg.N)
    nc = _get_nc(cfg)
    in_maps = host_prep(cfg, np.asarray(x, np.float32), np.asarray(adj),
                        np.asarray(W1, np.float32), np.asarray(a1, np.float32),
                        np.asarray(W2, np.float32), np.asarray(a2, np.float32))
    res = bass_utils.run_bass_kernel_spmd(nc, in_maps,
                                          core_ids=list(range(cfg.n_cores)))
    return host_finish(cfg, res.results)

